# revision 15
# baseline (speedup 1.0000x reference)
"""Causal self-attention (B=2, T=2048, C=1024, H=16) on 8 TRN2 NeuronCores.

Sharding: core c -> batch b = c//4, heads 4*(c%4) .. 4*(c%4)+3.
Each core computes q,k,v for its 4 heads (column-parallel qkv), causal
attention, and a partial output projection over its heads' rows of
w_proj (row-parallel). Host sums the 4 partials per batch and adds
b_proj (with the v-bias folded in: P@(V+bv)/l = P@V/l + bv, so
b_eff = b_proj + bv @ w_proj is added host-side for free).

Device schedule (per core, SPMD), designed so the PE never idles and
holds its high p-state:
  - Inputs DMA'd in consumption order (wqk, x[0:512], wv, x rest, wp),
    x chunked by 512-token blocks so qk matmuls start ~10us in.
  - qT,kT in [cols, tokens] layout; scores built transposed
    (S^T[j,i] = k_j . q_i). Head pairs at partition offsets 0/64 run
    their K=64 score matmuls concurrently on disjoint PE row groups.
  - exp on ACT only (one merged instruction per j-tile; ACT is the
    second-longest pole, everything else is kept off it). Causal mask:
    off-diagonal blocks skipped, matmul N-ranges below the diagonal,
    triu multiply (DVE) on the 128x128 diagonal blocks.
  - P@V accumulated as out^T[d,i] with V stationary; a ones-column in
    V yields the softmax denominator as PSUM row 64 for free.
  - normalization: reciprocal_approx_fast straight from PSUM ->
    gpsimd partition_broadcast -> DVE multiply into attT.
  - qkv-producer and projection units are emitted interleaved with
    attention blocks as PE filler; PSUM pools are stream-separated
    (scores 2x2 banks, PV accum 2x1, producer/proj 2x1 = 8 banks).
All matmuls in float32r (full PE rate at N>=256).
"""
import numpy as np

import concourse.bacc as bacc
import concourse.bass as bass
import concourse.mybir as mybir
import concourse.tile as tile
from concourse.bass_utils import run_bass_kernel_spmd

F32 = mybir.dt.float32
F32R = mybir.dt.float32r
AF = mybir.ActivationFunctionType

B, T, C = 2, 2048, 1024
H, DH = 16, 64
HPC = 4                    # heads per core
QKCOLS = 2 * HPC * DH      # 512 (q block 256 | k block 256)
VCOLS = HPC * DH           # 256
KC = C // 128              # 8 contraction chunks
TT = T // 128              # 16 token tiles
NB = T // 512              # 4 i-blocks


def build_nc():
    nc = bacc.Bacc("TRN2", target_bir_lowering=False, debug=False, num_devices=8)

    xT_d = nc.dram_tensor("xT", (C, T), F32R, kind="ExternalInput")
    wqk_d = nc.dram_tensor("wqk4", (4, 128, KC, 128), F32R, kind="ExternalInput")
    bqk_d = nc.dram_tensor("bqk", (128, 4), F32, kind="ExternalInput")
    wv_d = nc.dram_tensor("wv4", (128, KC, VCOLS), F32R, kind="ExternalInput")
    wp_d = nc.dram_tensor("wp4", (128, 2, C), F32R, kind="ExternalInput")
    triu_d = nc.dram_tensor("triu", (128, 128), F32R, kind="ExternalInput")
    ones_d = nc.dram_tensor("ones64", (128, 64), F32R, kind="ExternalInput")
    out_d = nc.dram_tensor("out", (T, C), F32, kind="ExternalOutput")

    with tile.TileContext(nc) as tc:
        with (
            tc.tile_pool(name="persist", bufs=1) as pp,
            tc.tile_pool(name="work", bufs=4) as pw,
            tc.tile_pool(name="nrm", bufs=3) as pn,
            tc.tile_pool(name="osb", bufs=3) as po,
            tc.tile_pool(name="ps_s", bufs=2, space="PSUM") as ps_s,
            tc.tile_pool(name="ps_oa", bufs=2, space="PSUM") as ps_oa,
            tc.tile_pool(name="ps_w", bufs=2, space="PSUM") as ps_w,
        ):
            # ---- persistent tiles ----
            triu = pp.tile([128, 128], F32R, tag="triu")
            ones64 = pp.tile([128, 64], F32R, tag="ones64")
            bqk_sb = pp.tile([128, 4], F32, tag="bqk")
            wqk_sb = pp.tile([128, 4, KC, 128], F32R, tag="wqk")
            wv_sb = pp.tile([128, KC, VCOLS], F32R, tag="wv")
            wp_sb = pp.tile([128, 2, C], F32R, tag="wp")
            xT_sb = pp.tile([128, KC, T], F32R, tag="xT")
            qkT = pp.tile([128, 4, T], F32R, tag="qkT")
            v_sb = pp.tile([128, TT, HPC, DH + 1], F32R, tag="v_sb")
            attT = pp.tile([128, 2, T], F32R, tag="attT")

            # ---- DMAs emitted in consumption/priority order ----
            nc.sync.dma_start(triu[:], triu_d.ap())
            nc.sync.dma_start(ones64[:], ones_d.ap())
            nc.sync.dma_start(bqk_sb[:], bqk_d.ap())
            for ct in (0, 2):
                nc.sync.dma_start(wqk_sb[:, ct], wqk_d.ap()[ct])
            xT_ap = xT_d.ap().rearrange("(kc p) t -> p kc t", p=128)
            nc.sync.dma_start(xT_sb[:, :, 0:512], xT_ap[:, :, 0:512])
            nc.sync.dma_start(wv_sb[:], wv_d.ap())
            for ct in (1, 3):
                nc.sync.dma_start(wqk_sb[:, ct], wqk_d.ap()[ct])
            for tb in range(1, 4):
                nc.sync.dma_start(
                    xT_sb[:, :, tb * 512 : (tb + 1) * 512],
                    xT_ap[:, :, tb * 512 : (tb + 1) * 512],
                )
            nc.sync.dma_start(wp_sb[:], wp_d.ap())

            # ACT exp-table pre-warm during the DMA head
            warm = pw.tile([1, 8], F32, tag="warm")
            nc.scalar.activation(warm[:], triu[0:1, 0:8], AF.Exp)

            # ones column of V (softmax denominator) written once
            nc.vector.tensor_copy(
                v_sb[:, :, :, DH],
                ones64[:].rearrange("p (a b) -> p a b", a=TT),
            )

            # ---- work units ----
            def qk_unit(ct, tb):
                with tc.high_priority():
                    ps = ps_w.tile([128, 512], F32, tag="w", name=f"qk{ct}_{tb}")
                    for kc in range(KC):
                        nc.tensor.matmul(
                            ps[:],
                            wqk_sb[:, ct, kc, :],
                            xT_sb[:, kc, tb * 512 : (tb + 1) * 512],
                            start=(kc == 0),
                            stop=(kc == KC - 1),
                        )
                    nc.vector.tensor_scalar_add(
                        qkT[:, ct, tb * 512 : (tb + 1) * 512], ps[:], bqk_sb[:, ct : ct + 1]
                    )

            def v_unit(tt):
                with tc.high_priority():
                    ps = ps_w.tile([128, 512], F32, tag="w", name=f"v{tt}")
                    for kc in range(KC):
                        nc.tensor.matmul(
                            ps[:, 0:VCOLS],
                            xT_sb[:, kc, tt * 128 : (tt + 1) * 128],
                            wv_sb[:, kc, :],
                            start=(kc == 0),
                            stop=(kc == KC - 1),
                        )
                    nc.vector.tensor_copy(
                        v_sb[:, tt, :, 0:DH],
                        ps[:, 0:VCOLS].rearrange("p (h d) -> p h d", h=HPC),
                    )

            def att_block(bi, g):
                ioff = bi * 512
                njt = 4 * bi + 4
                qT = [qkT[0:64, g, :], qkT[64:128, g, :]]
                kT = [qkT[0:64, 2 + g, :], qkT[64:128, 2 + g, :]]
                oa = [
                    ps_oa.tile([DH + 1, 512], F32, tag="oa", name=f"oa{bi}_{g}_{u}")
                    for u in range(2)
                ]
                with tc.high_priority():
                    for jt in range(njt):
                        d = jt - 4 * bi
                        so = d * 128 if d > 0 else 0
                        # u=1's cols start at 512 (not 512+so) so the score
                        # region [so : 1024-so] is contiguous and one merged
                        # exp instruction covers exactly the valid columns
                        ub = [so, 512]
                        ps = ps_s.tile([128, 1024], F32, tag="s")
                        ex = pw.tile([128, 1024], F32R, tag="exp")
                        for u in range(2):
                            nc.tensor.matmul(
                                ps[:, ub[u] : ub[u] + 512 - so],
                                kT[u][:, jt * 128 : (jt + 1) * 128],
                                qT[u][:, ioff + so : ioff + 512],
                                start=True,
                                stop=True,
                            )
                        nc.scalar.activation(
                            ex[:, so : 1024 - so], ps[:, so : 1024 - so], AF.Exp
                        )
                        if d >= 0:
                            for u in range(2):
                                nc.vector.tensor_mul(
                                    ex[:, ub[u] : ub[u] + 128],
                                    ex[:, ub[u] : ub[u] + 128],
                                    triu[:],
                                )
                        for u in range(2):
                            nc.tensor.matmul(
                                oa[u][:, so:512],
                                v_sb[:, jt, 2 * g + u, :],
                                ex[:, ub[u] : ub[u] + 512 - so],
                                start=(jt == 0),
                                stop=(jt == njt - 1),
                            )
                    # normalization for the head pair (reciprocal_approx_fast
                    # needs an SBUF partition-0 input on HW, hence lrow copy)
                    tail = bi == NB - 1 and g == 1
                    for u in range(2):
                        lrow = pn.tile([1, 512], F32, tag="lrow", name=f"lw{bi}_{g}_{u}")
                        if tail:
                            nc.scalar.copy(lrow[:], oa[u][DH : DH + 1, :])
                        else:
                            nc.vector.tensor_copy(lrow[:], oa[u][DH : DH + 1, :])
                        rst = pn.tile([1, 512], F32, tag="rst", name=f"rs{bi}_{g}_{u}")
                        nc.vector.reciprocal_approx_fast(rst[:], lrow[:])
                        rb = pn.tile([DH, 512], F32, tag="rb")
                        nc.gpsimd.partition_broadcast(rb[:], rst[:])
                        nc.vector.tensor_mul(
                            attT[64 * u : 64 * u + 64, g, ioff : ioff + 512],
                            oa[u][0:DH, :],
                            rb[:],
                        )

            def proj_unit(tt, half, tail=False):
                ps = ps_w.tile([128, 512], F32, tag="w", name=f"p{tt}_{half}")
                for kc2 in range(2):
                    nc.tensor.matmul(
                        ps[:],
                        attT[:, kc2, tt * 128 : (tt + 1) * 128],
                        wp_sb[:, kc2, half * 512 : (half + 1) * 512],
                        start=(kc2 == 0),
                        stop=(kc2 == 1),
                    )
                osb = po.tile([128, 512], F32, tag="osb")
                if tail:
                    nc.scalar.copy(osb[:], ps[:])  # ACT is idle at the tail
                else:
                    nc.vector.tensor_copy(osb[:], ps[:])
                nc.sync.dma_start(
                    out_d.ap()[tt * 128 : (tt + 1) * 128, half * 512 : (half + 1) * 512],
                    osb[:],
                )

            # ---- weave: attention paced, producer/proj as PE filler ----
            qk_unit(0, 0)
            qk_unit(2, 0)
            for tt in range(4):
                v_unit(tt)
            att_block(0, 0)
            qk_unit(1, 0)
            qk_unit(3, 0)
            att_block(0, 1)
            for bi in range(1, NB):
                qk_unit(0, bi)
                qk_unit(2, bi)
                for tt in range(4 * bi, 4 * bi + 4):
                    v_unit(tt)
                for tt in range(4 * (bi - 1), 4 * bi):
                    proj_unit(tt, 0)
                    proj_unit(tt, 1)
                att_block(bi, 0)
                qk_unit(1, bi)
                qk_unit(3, bi)
                att_block(bi, 1)
            for tt in range(12, 16):
                proj_unit(tt, 0, tail=True)
                proj_unit(tt, 1, tail=True)

    nc.compile()
    return nc


def make_core_inputs(x, w_qkv, b_qkv, w_proj, b_proj):
    """Per-core input maps (host-side sharding)."""
    x = np.asarray(x, dtype=np.float32)
    w_qkv = np.asarray(w_qkv, dtype=np.float32)
    b_qkv = np.asarray(b_qkv, dtype=np.float32)
    w_proj = np.asarray(w_proj, dtype=np.float32)

    consts = {
        "triu": np.triu(np.ones((128, 128), dtype=np.float32)),
        "ones64": np.ones((128, 64), dtype=np.float32),
    }
    in_maps = []
    for c in range(8):
        b = c // 4
        heads = [4 * (c % 4) + i for i in range(HPC)]
        qcols = np.concatenate([np.arange(64 * h, 64 * h + 64) for h in heads])
        wq = w_qkv[:, qcols] * 0.125
        bq = b_qkv[qcols] * 0.125
        wk = w_qkv[:, C + qcols]
        bk = b_qkv[C + qcols]
        wv = w_qkv[:, 2 * C + qcols]
        wqk = np.concatenate([wq, wk], axis=1)          # [C, 512]
        # wqk4[ct, p, kc, m] = wqk[kc*128+p, ct*128+m]
        wqk4 = np.ascontiguousarray(
            wqk.reshape(KC, 128, 4, 128).transpose(2, 1, 0, 3)
        )
        bqk = np.concatenate([bq, bk]).reshape(4, 128).T.copy()
        # wv4[p, kc, m] = wv[kc*128+p, m]
        wv4 = np.ascontiguousarray(wv.reshape(KC, 128, VCOLS).transpose(1, 0, 2))
        # wp4[p, kc2, n] = w_proj[qcols[kc2*128+p], n]
        wp4 = np.ascontiguousarray(
            w_proj[qcols, :].reshape(2, 128, C).transpose(1, 0, 2)
        )
        in_maps.append({
            "xT": np.ascontiguousarray(x[b].T),
            "wqk4": wqk4,
            "bqk": bqk,
            "wv4": wv4,
            "wp4": wp4,
            **consts,
        })
    return in_maps


_NC_CACHE = []


def kernel(x, w_qkv, b_qkv, w_proj, b_proj):
    if not _NC_CACHE:
        _NC_CACHE.append(build_nc())
    nc = _NC_CACHE[0]
    in_maps = make_core_inputs(x, w_qkv, b_qkv, w_proj, b_proj)
    res = run_bass_kernel_spmd(nc, in_maps, list(range(8)))
    w_proj = np.asarray(w_proj, dtype=np.float64)
    bv = np.asarray(b_qkv, dtype=np.float64)[2 * C : 3 * C]
    b_eff = (np.asarray(b_proj, dtype=np.float64) + bv @ w_proj).astype(np.float32)
    out = np.empty((B, T, C), dtype=np.float32)
    for b in range(B):
        acc = res.results[4 * b]["out"].astype(np.float32).copy()
        for c in range(4 * b + 1, 4 * b + 4):
            acc += res.results[c]["out"]
        out[b] = acc + b_eff
    return out


# revision 17
# speedup vs baseline: 1.1175x; 1.1175x over previous
"""Causal self-attention (B=2, T=2048, C=1024, H=16) on 8 TRN2 NeuronCores.

Sharding: core c -> batch b = c//4, heads 4*(c%4) .. 4*(c%4)+3.
Each core computes q,k,v for its 4 heads (column-parallel qkv), causal
attention, and a partial output projection over its heads' rows of
w_proj (row-parallel). Host sums the 4 partials per batch and adds
b_proj (with the v-bias folded in: P@(V+bv)/l = P@V/l + bv, so
b_eff = b_proj + bv @ w_proj is added host-side for free).

Device schedule (per core, SPMD), designed so the PE never idles and
holds its high p-state:
  - Inputs DMA'd in consumption order (wqk, x[0:512], wv, x rest, wp),
    x chunked by 512-token blocks so qk matmuls start ~10us in.
  - qT,kT in [cols, tokens] layout; scores built transposed
    (S^T[j,i] = k_j . q_i). Head pairs at partition offsets 0/64 run
    their K=64 score matmuls concurrently on disjoint PE row groups.
  - exp on ACT only (one merged instruction per j-tile; ACT is the
    second-longest pole, everything else is kept off it). Causal mask:
    off-diagonal blocks skipped, matmul N-ranges below the diagonal,
    triu multiply (DVE) on the 128x128 diagonal blocks.
  - P@V accumulated as out^T[d,i] with V stationary; a ones-column in
    V yields the softmax denominator as PSUM row 64 for free.
  - normalization: reciprocal_approx_fast straight from PSUM ->
    gpsimd partition_broadcast -> DVE multiply into attT.
  - qkv-producer and projection units are emitted interleaved with
    attention blocks as PE filler; PSUM pools are stream-separated
    (scores 2x2 banks, PV accum 2x1, producer/proj 2x1 = 8 banks).
All matmuls in float32r (full PE rate at N>=256).
"""
import numpy as np

import concourse.bacc as bacc
import concourse.bass as bass
import concourse.mybir as mybir
import concourse.tile as tile
from concourse.bass_utils import run_bass_kernel_spmd

F32 = mybir.dt.float32
F32R = mybir.dt.float32r
AF = mybir.ActivationFunctionType

B, T, C = 2, 2048, 1024
H, DH = 16, 64
HPC = 4                    # heads per core
QKCOLS = 2 * HPC * DH      # 512 (q block 256 | k block 256)
VCOLS = HPC * DH           # 256
KC = C // 128              # 8 contraction chunks
TT = T // 128              # 16 token tiles
NB = T // 512              # 4 i-blocks


def build_nc():
    nc = bacc.Bacc("TRN2", target_bir_lowering=False, debug=False, num_devices=8)

    xT_d = nc.dram_tensor("xT", (C, T), F32R, kind="ExternalInput")
    wqk_d = nc.dram_tensor("wqk4", (4, 128, KC, 128), F32R, kind="ExternalInput")
    bqk_d = nc.dram_tensor("bqk", (128, 4), F32, kind="ExternalInput")
    wv_d = nc.dram_tensor("wv4", (128, KC, VCOLS), F32R, kind="ExternalInput")
    wp_d = nc.dram_tensor("wp4", (128, 2, C), F32R, kind="ExternalInput")
    triu_d = nc.dram_tensor("triu", (128, 128), F32R, kind="ExternalInput")
    ones_d = nc.dram_tensor("ones64", (128, 64), F32R, kind="ExternalInput")
    out_d = nc.dram_tensor("out", (T, C), F32, kind="ExternalOutput")

    with tile.TileContext(nc) as tc:
        with (
            tc.tile_pool(name="persist", bufs=1) as pp,
            tc.tile_pool(name="work", bufs=4) as pw,
            tc.tile_pool(name="nrm", bufs=3) as pn,
            tc.tile_pool(name="osb", bufs=3) as po,
            tc.tile_pool(name="ps_s", bufs=2, space="PSUM") as ps_s,
            tc.tile_pool(name="ps_oa", bufs=2, space="PSUM") as ps_oa,
            tc.tile_pool(name="ps_w", bufs=2, space="PSUM") as ps_w,
        ):
            # ---- persistent tiles ----
            triu = pp.tile([128, 128], F32R, tag="triu")
            ones64 = pp.tile([128, 64], F32R, tag="ones64")
            bqk_sb = pp.tile([128, 4], F32, tag="bqk")
            wqk_sb = pp.tile([128, 4, KC, 128], F32R, tag="wqk")
            wv_sb = pp.tile([128, KC, VCOLS], F32R, tag="wv")
            wp_sb = pp.tile([128, 2, C], F32R, tag="wp")
            xT_sb = pp.tile([128, KC, T], F32R, tag="xT")
            qkT = pp.tile([128, 4, T], F32R, tag="qkT")
            v_sb = pp.tile([128, TT, HPC, DH + 1], F32R, tag="v_sb")
            attT = pp.tile([128, 2, T], F32R, tag="attT")

            # ---- DMAs emitted in consumption/priority order ----
            nc.sync.dma_start(triu[:], triu_d.ap())
            nc.sync.dma_start(ones64[:], ones_d.ap())
            nc.sync.dma_start(bqk_sb[:], bqk_d.ap())
            for ct in (0, 2):
                nc.sync.dma_start(wqk_sb[:, ct], wqk_d.ap()[ct])
            xT_ap = xT_d.ap().rearrange("(kc p) t -> p kc t", p=128)
            nc.sync.dma_start(xT_sb[:, :, 0:512], xT_ap[:, :, 0:512])
            nc.sync.dma_start(wv_sb[:], wv_d.ap())
            for ct in (1, 3):
                nc.sync.dma_start(wqk_sb[:, ct], wqk_d.ap()[ct])
            for tb in range(1, 4):
                nc.sync.dma_start(
                    xT_sb[:, :, tb * 512 : (tb + 1) * 512],
                    xT_ap[:, :, tb * 512 : (tb + 1) * 512],
                )
            nc.sync.dma_start(wp_sb[:], wp_d.ap())

            # ACT exp-table pre-warm during the DMA head
            warm = pw.tile([1, 8], F32, tag="warm")
            nc.scalar.activation(warm[:], triu[0:1, 0:8], AF.Exp)

            # ones column of V (softmax denominator) written once
            nc.vector.tensor_copy(
                v_sb[:, :, :, DH],
                ones64[:].rearrange("p (a b) -> p a b", a=TT),
            )

            # ---- work units ----
            def qk_unit(ct, tb):
                ps = ps_w.tile([128, 512], F32, tag="w", name=f"qk{ct}_{tb}")
                for kc in range(KC):
                    nc.tensor.matmul(
                        ps[:],
                        wqk_sb[:, ct, kc, :],
                        xT_sb[:, kc, tb * 512 : (tb + 1) * 512],
                        start=(kc == 0),
                        stop=(kc == KC - 1),
                    )
                nc.vector.tensor_scalar_add(
                    qkT[:, ct, tb * 512 : (tb + 1) * 512], ps[:], bqk_sb[:, ct : ct + 1]
                )

            def v_unit(tt):
                ps = ps_w.tile([128, 512], F32, tag="w", name=f"v{tt}")
                for kc in range(KC):
                    nc.tensor.matmul(
                        ps[:, 0:VCOLS],
                        xT_sb[:, kc, tt * 128 : (tt + 1) * 128],
                        wv_sb[:, kc, :],
                        start=(kc == 0),
                        stop=(kc == KC - 1),
                    )
                nc.vector.tensor_copy(
                    v_sb[:, tt, :, 0:DH],
                    ps[:, 0:VCOLS].rearrange("p (h d) -> p h d", h=HPC),
                )

            def att_block(bi, g):
                ioff = bi * 512
                njt = 4 * bi + 4
                qT = [qkT[0:64, g, :], qkT[64:128, g, :]]
                kT = [qkT[0:64, 2 + g, :], qkT[64:128, 2 + g, :]]
                oa = [
                    ps_oa.tile([DH + 1, 512], F32, tag="oa", name=f"oa{bi}_{g}_{u}")
                    for u in range(2)
                ]
                for jt in range(njt):
                    d = jt - 4 * bi
                    so = d * 128 if d > 0 else 0
                    # u=1's cols start at 512 (not 512+so) so the score
                    # region [so : 1024-so] is contiguous and one merged
                    # exp instruction covers exactly the valid columns
                    ub = [so, 512]
                    ps = ps_s.tile([128, 1024], F32, tag="s")
                    ex = pw.tile([128, 1024], F32R, tag="exp")
                    for u in range(2):
                        nc.tensor.matmul(
                            ps[:, ub[u] : ub[u] + 512 - so],
                            kT[u][:, jt * 128 : (jt + 1) * 128],
                            qT[u][:, ioff + so : ioff + 512],
                            start=True,
                            stop=True,
                        )
                    nc.scalar.activation(
                        ex[:, so : 1024 - so], ps[:, so : 1024 - so], AF.Exp
                    )
                    if d >= 0:
                        for u in range(2):
                            nc.vector.tensor_mul(
                                ex[:, ub[u] : ub[u] + 128],
                                ex[:, ub[u] : ub[u] + 128],
                                triu[:],
                            )
                    for u in range(2):
                        nc.tensor.matmul(
                            oa[u][:, so:512],
                            v_sb[:, jt, 2 * g + u, :],
                            ex[:, ub[u] : ub[u] + 512 - so],
                            start=(jt == 0),
                            stop=(jt == njt - 1),
                        )
                # normalization for the head pair (reciprocal_approx_fast
                # needs an SBUF partition-0 input on HW, hence lrow copy)
                tail = bi == NB - 1 and g == 1
                for u in range(2):
                    lrow = pn.tile([1, 512], F32, tag="lrow", name=f"lw{bi}_{g}_{u}")
                    if tail:
                        nc.scalar.copy(lrow[:], oa[u][DH : DH + 1, :])
                    else:
                        nc.vector.tensor_copy(lrow[:], oa[u][DH : DH + 1, :])
                    rst = pn.tile([1, 512], F32, tag="rst", name=f"rs{bi}_{g}_{u}")
                    nc.vector.reciprocal_approx_fast(rst[:], lrow[:])
                    rb = pn.tile([DH, 512], F32, tag="rb")
                    nc.gpsimd.partition_broadcast(rb[:], rst[:])
                    nc.vector.tensor_mul(
                        attT[64 * u : 64 * u + 64, g, ioff : ioff + 512],
                        oa[u][0:DH, :],
                        rb[:],
                    )

            def proj_unit(tt, half, tail=False):
                ps = ps_w.tile([128, 512], F32, tag="w", name=f"p{tt}_{half}")
                for kc2 in range(2):
                    nc.tensor.matmul(
                        ps[:],
                        attT[:, kc2, tt * 128 : (tt + 1) * 128],
                        wp_sb[:, kc2, half * 512 : (half + 1) * 512],
                        start=(kc2 == 0),
                        stop=(kc2 == 1),
                    )
                osb = po.tile([128, 512], F32, tag="osb")
                if tail:
                    nc.scalar.copy(osb[:], ps[:])  # ACT is idle at the tail
                else:
                    nc.vector.tensor_copy(osb[:], ps[:])
                nc.sync.dma_start(
                    out_d.ap()[tt * 128 : (tt + 1) * 128, half * 512 : (half + 1) * 512],
                    osb[:],
                )

            # ---- weave: attention paced, producer/proj as PE filler ----
            qk_unit(0, 0)
            qk_unit(2, 0)
            for tt in range(4):
                v_unit(tt)
            att_block(0, 0)
            qk_unit(1, 0)
            qk_unit(3, 0)
            att_block(0, 1)
            for bi in range(1, NB):
                qk_unit(0, bi)
                qk_unit(2, bi)
                for tt in range(4 * bi, 4 * bi + 4):
                    v_unit(tt)
                for tt in range(4 * (bi - 1), 4 * bi):
                    proj_unit(tt, 0)
                    proj_unit(tt, 1)
                att_block(bi, 0)
                qk_unit(1, bi)
                qk_unit(3, bi)
                att_block(bi, 1)
            for tt in range(12, 16):
                proj_unit(tt, 0, tail=True)
                proj_unit(tt, 1, tail=True)

    nc.compile()
    return nc


def make_core_inputs(x, w_qkv, b_qkv, w_proj, b_proj):
    """Per-core input maps (host-side sharding)."""
    x = np.asarray(x, dtype=np.float32)
    w_qkv = np.asarray(w_qkv, dtype=np.float32)
    b_qkv = np.asarray(b_qkv, dtype=np.float32)
    w_proj = np.asarray(w_proj, dtype=np.float32)

    consts = {
        "triu": np.triu(np.ones((128, 128), dtype=np.float32)),
        "ones64": np.ones((128, 64), dtype=np.float32),
    }
    in_maps = []
    for c in range(8):
        b = c // 4
        heads = [4 * (c % 4) + i for i in range(HPC)]
        qcols = np.concatenate([np.arange(64 * h, 64 * h + 64) for h in heads])
        wq = w_qkv[:, qcols] * 0.125
        bq = b_qkv[qcols] * 0.125
        wk = w_qkv[:, C + qcols]
        bk = b_qkv[C + qcols]
        wv = w_qkv[:, 2 * C + qcols]
        wqk = np.concatenate([wq, wk], axis=1)          # [C, 512]
        # wqk4[ct, p, kc, m] = wqk[kc*128+p, ct*128+m]
        wqk4 = np.ascontiguousarray(
            wqk.reshape(KC, 128, 4, 128).transpose(2, 1, 0, 3)
        )
        bqk = np.concatenate([bq, bk]).reshape(4, 128).T.copy()
        # wv4[p, kc, m] = wv[kc*128+p, m]
        wv4 = np.ascontiguousarray(wv.reshape(KC, 128, VCOLS).transpose(1, 0, 2))
        # wp4[p, kc2, n] = w_proj[qcols[kc2*128+p], n]
        wp4 = np.ascontiguousarray(
            w_proj[qcols, :].reshape(2, 128, C).transpose(1, 0, 2)
        )
        in_maps.append({
            "xT": np.ascontiguousarray(x[b].T),
            "wqk4": wqk4,
            "bqk": bqk,
            "wv4": wv4,
            "wp4": wp4,
            **consts,
        })
    return in_maps


_NC_CACHE = []


def kernel(x, w_qkv, b_qkv, w_proj, b_proj):
    if not _NC_CACHE:
        _NC_CACHE.append(build_nc())
    nc = _NC_CACHE[0]
    in_maps = make_core_inputs(x, w_qkv, b_qkv, w_proj, b_proj)
    res = run_bass_kernel_spmd(nc, in_maps, list(range(8)))
    w_proj = np.asarray(w_proj, dtype=np.float64)
    bv = np.asarray(b_qkv, dtype=np.float64)[2 * C : 3 * C]
    b_eff = (np.asarray(b_proj, dtype=np.float64) + bv @ w_proj).astype(np.float32)
    out = np.empty((B, T, C), dtype=np.float32)
    for b in range(B):
        acc = res.results[4 * b]["out"].astype(np.float32).copy()
        for c in range(4 * b + 1, 4 * b + 4):
            acc += res.results[c]["out"]
        out[b] = acc + b_eff
    return out
